# revision 11
# baseline (speedup 1.0000x reference)
"""Trainium2 Bass kernel for BasicMGU (nn_BasicMGU_53386443489965).

Math (per reference):
    xz = x @ W_k ; xh = x @ W_u
    f_t = sigmoid(xz_t + h @ W_r + b_r)
    c_t = tanh(xh_t + (h*f_t) @ W_ur + b_ur)
    h   = (1-f_t)*h + f_t*c_t        -> return final h  [B, U]

Sharding: data-parallel over batch across 8 cores (B=64 -> 8 per core),
weights replicated.

Per-core design:
  Phase 1 (projections): two GEMMs in fp32r (full PE rate at N=512),
  producing xzT/xhT in DRAM stored time-major *transposed* [T, U, B]
  with biases folded in.
  Phase 2 (recurrence): state kept transposed hT [U(part), B(free)].
  Both per-step matmuls run weight-stationary (lhsT = 128x128 weight
  tile in bf16 -> fast weight load, rhs = state in bf16, N=B=8), so no
  per-step transposes are needed and PSUM outputs stay transposed.
  Elementwise/activations run on [128, 4, 8] tiles (128 partitions).
"""

import os
import sys
import types

sys.path.insert(0, "/opt/trn_rl_repo")

import numpy as np
import ml_dtypes

import concourse.bass as bass
import concourse.mybir as mybir
import concourse.tile as tile
from concourse import bacc
from concourse.bass_utils import run_bass_kernel_spmd

B, T, D, U = 64, 1024, 512, 512
NCORES = 8
BL = B // NCORES          # batch per core
S = int(os.environ.get("MGU_S", 16))  # recurrence steps per hw-loop iteration
KC = D // 128             # contraction chunks
MC = U // 128             # output-unit chunks
PCOLS = 512               # projection (t,b) columns per block
NBLK = T * BL // PCOLS

F32 = mybir.dt.float32
F32R = mybir.dt.float32r
BF16 = mybir.dt.bfloat16

LAST_EXEC_NS = None


def _install_trace_shim():
    """Make `antenv.axon_hooks` importable so trace=True degrades gracefully
    (and, where the axon .so is present, actually captures NTFF profiles)."""
    if "antenv.axon_hooks" in sys.modules:
        return
    mod = types.ModuleType("antenv.axon_hooks")
    holder = [None]
    mod.set_axon_ntff_profile_hook = lambda h: holder.__setitem__(0, h)
    mod.get_axon_ntff_profile_hook = lambda: holder[0]
    sys.modules["antenv.axon_hooks"] = mod
    try:
        if "/root/.axon_site" not in sys.path:
            sys.path.append("/root/.axon_site")
        from trn_agent_boot.trn_boot import _ntff_profile_via_ctypes

        hook = _ntff_profile_via_ctypes("/opt/axon/libaxon_pjrt.so")
        if hook is not None:
            mod.set_axon_ntff_profile_hook(hook)
    except Exception:
        pass


def _build():
    nc = bacc.Bacc("TRN2")

    xT = nc.dram_tensor("xT", [D, T * BL], F32, kind="ExternalInput")
    Wk = nc.dram_tensor("Wk", [D, U], F32, kind="ExternalInput")
    Wu = nc.dram_tensor("Wu", [D, U], F32, kind="ExternalInput")
    Wr = nc.dram_tensor("Wr", [U, U], BF16, kind="ExternalInput")
    Wur = nc.dram_tensor("Wur", [U, U], BF16, kind="ExternalInput")
    br = nc.dram_tensor("br", [U], F32, kind="ExternalInput")
    bur = nc.dram_tensor("bur", [U], F32, kind="ExternalInput")
    hT_out = nc.dram_tensor("hT_out", [128, MC, BL], F32, kind="ExternalOutput")
    dbg = os.environ.get("MGU_DEBUG")
    proj_kind = {"kind": "ExternalOutput"} if dbg else {}
    xzT_d = nc.dram_tensor("xzT_d", [T, U, BL], F32, **proj_kind)
    xhT_d = nc.dram_tensor("xhT_d", [T, U, BL], F32, **proj_kind)
    dbg2 = os.environ.get("MGU_DEBUG2")
    if dbg2:
        nd = int(os.environ.get("MGU_TSTEPS", T))
        f_dbg = nc.dram_tensor("f_dbg", [nd, 128, MC, BL], F32, kind="ExternalOutput")
        c_dbg = nc.dram_tensor("c_dbg", [nd, 128, MC, BL], F32, kind="ExternalOutput")
        h_dbg = nc.dram_tensor("h_dbg", [nd, 128, MC, BL], F32, kind="ExternalOutput")
        hf_dbg = nc.dram_tensor("hf_dbg", [nd, 128, MC, BL], BF16, kind="ExternalOutput")
        z2_dbg = nc.dram_tensor("z2_dbg", [nd, 128, MC, BL], F32, kind="ExternalOutput")

    ID = mybir.ActivationFunctionType.Identity
    SIG = mybir.ActivationFunctionType.Sigmoid
    TANH = mybir.ActivationFunctionType.Tanh

    with tile.TileContext(nc) as tc:
        with tc.tile_pool(name="consts", bufs=1) as consts:
            # fp32r matmul inputs must be produced by a compute op (walrus
            # verifier rejects DMA-written fp32r operands), so stage via f32
            # tiles and round with a DVE copy.
            Wk_st = consts.tile([128, KC, U], F32)
            nc.sync.dma_start(Wk_st, Wk[:, :].rearrange("(c p) u -> p c u", p=128))
            Wk_sb = consts.tile([128, KC, U], F32R)
            nc.vector.tensor_copy(Wk_sb, Wk_st)
            Wu_st = consts.tile([128, KC, U], F32)
            nc.sync.dma_start(Wu_st, Wu[:, :].rearrange("(c p) u -> p c u", p=128))
            Wu_sb = consts.tile([128, KC, U], F32R)
            nc.vector.tensor_copy(Wu_sb, Wu_st)
            Wr_sb = consts.tile([128, MC, U], BF16)
            nc.sync.dma_start(Wr_sb, Wr[:, :].rearrange("(c p) u -> p c u", p=128))
            Wur_sb = consts.tile([128, MC, U], BF16)
            nc.sync.dma_start(Wur_sb, Wur[:, :].rearrange("(c p) u -> p c u", p=128))
            br_sb = consts.tile([128, MC], F32)
            nc.sync.dma_start(br_sb, br[:].rearrange("(c p) -> p c", p=128))
            bur_sb = consts.tile([128, MC], F32)
            nc.sync.dma_start(bur_sb, bur[:].rearrange("(c p) -> p c", p=128))

            hTf = consts.tile([128, MC, BL], F32)
            nc.vector.memset(hTf, 0.0)
            hTb = consts.tile([128, MC, BL], BF16)
            nc.vector.memset(hTb, 0.0)

            # ---------------- Phase 1: projections ----------------
            with (
                tc.tile_pool(name="proj_in", bufs=2) as pin,
                tc.tile_pool(name="proj_ps", bufs=4, space="PSUM") as pps,
                tc.tile_pool(name="proj_out", bufs=4) as pout,
            ):
                tblk = PCOLS // BL  # timesteps per column block
                for j in range(NBLK):
                    xT_st = pin.tile([128, KC, PCOLS], F32, tag="xT_st")
                    nc.sync.dma_start(
                        xT_st,
                        xT[:, j * PCOLS : (j + 1) * PCOLS].rearrange(
                            "(c p) n -> p c n", p=128
                        ),
                    )
                    xT_sb = pin.tile([128, KC, PCOLS], F32R, tag="xT_r")
                    nc.vector.tensor_copy(xT_sb, xT_st)
                    for W_sb, bias_sb, dst in (
                        (Wk_sb, br_sb, xzT_d),
                        (Wu_sb, bur_sb, xhT_d),
                    ):
                        for m in range(MC):
                            ps = pps.tile([128, PCOLS], F32)
                            for k in range(KC):
                                nc.tensor.matmul(
                                    ps,
                                    W_sb[:, k, m * 128 : (m + 1) * 128],
                                    xT_sb[:, k, :],
                                    start=(k == 0),
                                    stop=(k == KC - 1),
                                )
                            o = pout.tile([128, tblk, BL], F32)
                            nc.scalar.activation(o, ps, ID, bias=bias_sb[:, m : m + 1])
                            nc.sync.dma_start(
                                dst[
                                    j * tblk : (j + 1) * tblk,
                                    m * 128 : (m + 1) * 128,
                                    :,
                                ].rearrange("t u b -> u t b"),
                                o,
                            )

            # ---------------- Phase 2: recurrence ----------------
            with (
                tc.tile_pool(name="rec_in", bufs=2) as rin,
                tc.tile_pool(name="rec_ps", bufs=2, space="PSUM") as rps,
                tc.tile_pool(name="rec_tmp", bufs=3) as rtmp,
            ):
                t_total = int(os.environ.get("MGU_TSTEPS", T))
                with tc.For_i(0, t_total, S) as it:
                    xz_sb = rin.tile([128, S, MC, BL], F32, tag="xz")
                    nc.sync.dma_start(
                        xz_sb,
                        xzT_d[bass.ds(it, S), :, :].rearrange(
                            "t (c p) b -> p t c b", p=128
                        ),
                    )
                    xh_sb = rin.tile([128, S, MC, BL], F32, tag="xh")
                    nc.sync.dma_start(
                        xh_sb,
                        xhT_d[bass.ds(it, S), :, :].rearrange(
                            "t (c p) b -> p t c b", p=128
                        ),
                    )
                    for s in range(S):
                        ps1 = rps.tile([128, MC, BL], F32, tag="ps1")
                        for m in range(MC):
                            for k in range(KC):
                                nc.tensor.matmul(
                                    ps1[:, m, :],
                                    Wr_sb[:, k, m * 128 : (m + 1) * 128],
                                    hTb[:, k, :],
                                    start=(k == 0),
                                    stop=(k == KC - 1),
                                )
                        fT = rtmp.tile([128, MC, BL], F32, tag="fT")
                        hfb = rtmp.tile([128, MC, BL], BF16, tag="hfb")
                        z1 = rtmp.tile([128, MC, BL], F32, tag="z1")
                        for m in range(MC):
                            nc.vector.tensor_add(
                                z1[:, m, :], ps1[:, m, :], xz_sb[:, s, m, :]
                            )
                            nc.scalar.activation(fT[:, m, :], z1[:, m, :], SIG)
                            nc.vector.tensor_mul(
                                hfb[:, m, :], hTf[:, m, :], fT[:, m, :]
                            )
                        ps2 = rps.tile([128, MC, BL], F32, tag="ps2")
                        for m in range(MC):
                            for k in range(KC):
                                nc.tensor.matmul(
                                    ps2[:, m, :],
                                    Wur_sb[:, k, m * 128 : (m + 1) * 128],
                                    hfb[:, k, :],
                                    start=(k == 0),
                                    stop=(k == KC - 1),
                                )
                        cT = rtmp.tile([128, MC, BL], F32, tag="cT")
                        z2 = rtmp.tile([128, MC, BL], F32, tag="z2")
                        for m in range(MC):
                            nc.vector.tensor_add(
                                z2[:, m, :], ps2[:, m, :], xh_sb[:, s, m, :]
                            )
                            nc.scalar.activation(cT[:, m, :], z2[:, m, :], TANH)
                        if dbg2:
                            nc.sync.dma_start(f_dbg[bass.ds(it + s, 1)], fT)
                            nc.sync.dma_start(c_dbg[bass.ds(it + s, 1)], cT)
                            nc.sync.dma_start(hf_dbg[bass.ds(it + s, 1)], hfb)
                            nc.sync.dma_start(z2_dbg[bass.ds(it + s, 1)], z2)
                        nc.vector.tensor_sub(cT, cT, hTf)
                        nc.vector.tensor_mul(cT, cT, fT)
                        nc.vector.tensor_add(hTf, hTf, cT)
                        nc.scalar.copy(hTb, hTf)
                        if dbg2:
                            nc.sync.dma_start(h_dbg[bass.ds(it + s, 1)], hTf)

            nc.sync.dma_start(hT_out[:, :, :], hTf)

    nc.compile()
    return nc


_NC_CACHE = None


def kernel(x, W_k, W_r, b_r, W_u, W_ur, b_ur):
    global _NC_CACHE, LAST_EXEC_NS
    _install_trace_shim()
    if _NC_CACHE is None:
        _NC_CACHE = _build()
    nc = _NC_CACHE

    x = np.ascontiguousarray(np.asarray(x, dtype=np.float32))
    Wr_b = np.asarray(W_r, dtype=np.float32).astype(ml_dtypes.bfloat16)
    Wur_b = np.asarray(W_ur, dtype=np.float32).astype(ml_dtypes.bfloat16)
    Wk_f = np.ascontiguousarray(np.asarray(W_k, dtype=np.float32))
    Wu_f = np.ascontiguousarray(np.asarray(W_u, dtype=np.float32))
    br_f = np.ascontiguousarray(np.asarray(b_r, dtype=np.float32))
    bur_f = np.ascontiguousarray(np.asarray(b_ur, dtype=np.float32))

    in_maps = []
    for c in range(NCORES):
        xc = x[c * BL : (c + 1) * BL]  # [BL, T, D]
        xTc = np.ascontiguousarray(xc.transpose(2, 1, 0).reshape(D, T * BL))
        in_maps.append(
            {
                "xT": xTc,
                "Wk": Wk_f,
                "Wu": Wu_f,
                "Wr": Wr_b,
                "Wur": Wur_b,
                "br": br_f,
                "bur": bur_f,
            }
        )

    trace = bool(os.environ.get("BASS_TRACE"))
    res = run_bass_kernel_spmd(
        nc, in_maps, core_ids=list(range(NCORES)), trace=trace
    )
    LAST_EXEC_NS = res.exec_time_ns

    out = np.empty((B, U), dtype=np.float32)
    for c in range(NCORES):
        hT = res.results[c]["hT_out"]  # [128, MC, BL]
        out[c * BL : (c + 1) * BL] = hT.transpose(2, 1, 0).reshape(BL, U)
    return out


# revision 12
# speedup vs baseline: 1.2753x; 1.2753x over previous
"""Trainium2 Bass kernel for BasicMGU (nn_BasicMGU_53386443489965).

Math (per reference):
    xz = x @ W_k ; xh = x @ W_u
    f_t = sigmoid(xz_t + h @ W_r + b_r)
    c_t = tanh(xh_t + (h*f_t) @ W_ur + b_ur)
    h   = (1-f_t)*h + f_t*c_t        -> return final h  [B, U]

Sharding: data-parallel over batch across 8 cores (B=64 -> 8 per core),
weights replicated.

Per-core design:
  Phase 1 (projections): two GEMMs in fp32r (full PE rate at N=512),
  producing xzT/xhT in DRAM pre-swizzled into the exact per-chunk SBUF
  layout the recurrence consumes (contiguous 512B+ runs per DMA
  descriptor), biases folded in.
  Phase 2 (recurrence): state kept transposed hT [U(part), B(free)].
  Both per-step matmuls run weight-stationary (lhsT = 128x128 weight
  tile in bf16 -> fast weight load, rhs = state in bf16, N=B=8), so no
  per-step transposes are needed and PSUM outputs stay transposed.
  Accumulation groups are kept consecutive per PSUM slice (m-outer,
  k-inner) - interleaving groups gives wrong results on HW.
  Elementwise/activations run on [128, ...] tiles (128 partitions).
"""

import os
import sys
import types

sys.path.insert(0, "/opt/trn_rl_repo")

import numpy as np
import ml_dtypes

import concourse.bass as bass
import concourse.mybir as mybir
import concourse.tile as tile
from concourse import bacc
from concourse.bass_utils import run_bass_kernel_spmd

B, T, D, U = 64, 1024, 512, 512
NCORES = 8
BL = B // NCORES          # batch per core
S = int(os.environ.get("MGU_S", 32))  # recurrence steps per hw-loop iteration
KC = D // 128             # contraction chunks
MC = U // 128             # output-unit chunks
PCOLS = 512               # projection (t,b) columns per block
NBLK = T * BL // PCOLS
NW = S * BL               # free width of one swizzled chunk slab

F32 = mybir.dt.float32
F32R = mybir.dt.float32r
BF16 = mybir.dt.bfloat16

LAST_EXEC_NS = None


def _install_trace_shim():
    """Make `antenv.axon_hooks` importable so trace=True degrades gracefully
    (and, where the axon .so is present, actually captures NTFF profiles)."""
    if "antenv.axon_hooks" in sys.modules:
        return
    mod = types.ModuleType("antenv.axon_hooks")
    holder = [None]
    mod.set_axon_ntff_profile_hook = lambda h: holder.__setitem__(0, h)
    mod.get_axon_ntff_profile_hook = lambda: holder[0]
    sys.modules["antenv.axon_hooks"] = mod
    try:
        if "/root/.axon_site" not in sys.path:
            sys.path.append("/root/.axon_site")
        from trn_agent_boot.trn_boot import _ntff_profile_via_ctypes

        hook = _ntff_profile_via_ctypes("/opt/axon/libaxon_pjrt.so")
        if hook is not None:
            mod.set_axon_ntff_profile_hook(hook)
    except Exception:
        pass


def _build():
    nc = bacc.Bacc("TRN2")

    t_total = int(os.environ.get("MGU_TSTEPS", T))
    nch = t_total // S

    xT = nc.dram_tensor("xT", [D, T * BL], F32, kind="ExternalInput")
    Wk = nc.dram_tensor("Wk", [D, U], F32, kind="ExternalInput")
    Wu = nc.dram_tensor("Wu", [D, U], F32, kind="ExternalInput")
    Wr = nc.dram_tensor("Wr", [U, U], BF16, kind="ExternalInput")
    Wur = nc.dram_tensor("Wur", [U, U], BF16, kind="ExternalInput")
    br = nc.dram_tensor("br", [U], F32, kind="ExternalInput")
    bur = nc.dram_tensor("bur", [U], F32, kind="ExternalInput")
    hT_out = nc.dram_tensor("hT_out", [128, MC, BL], F32, kind="ExternalOutput")
    # Swizzled step-input slabs: [chunk, m, partition(u%128), (s b)]
    xzT_d = nc.dram_tensor("xzT_d", [T // S, MC, 128, NW], F32)
    xhT_d = nc.dram_tensor("xhT_d", [T // S, MC, 128, NW], F32)

    ID = mybir.ActivationFunctionType.Identity
    SIG = mybir.ActivationFunctionType.Sigmoid
    TANH = mybir.ActivationFunctionType.Tanh

    with tile.TileContext(nc) as tc:
        with tc.tile_pool(name="consts", bufs=1) as consts:
            # fp32r matmul inputs must be produced by a compute op (walrus
            # verifier rejects DMA-written fp32r operands), so stage via f32
            # tiles and round with a DVE copy.
            Wk_st = consts.tile([128, KC, U], F32)
            nc.sync.dma_start(Wk_st, Wk[:, :].rearrange("(c p) u -> p c u", p=128))
            Wk_sb = consts.tile([128, KC, U], F32R)
            nc.vector.tensor_copy(Wk_sb, Wk_st)
            Wu_st = consts.tile([128, KC, U], F32)
            nc.sync.dma_start(Wu_st, Wu[:, :].rearrange("(c p) u -> p c u", p=128))
            Wu_sb = consts.tile([128, KC, U], F32R)
            nc.vector.tensor_copy(Wu_sb, Wu_st)
            Wr_sb = consts.tile([128, MC, U], BF16)
            nc.sync.dma_start(Wr_sb, Wr[:, :].rearrange("(c p) u -> p c u", p=128))
            Wur_sb = consts.tile([128, MC, U], BF16)
            nc.sync.dma_start(Wur_sb, Wur[:, :].rearrange("(c p) u -> p c u", p=128))
            br_sb = consts.tile([128, MC], F32)
            nc.sync.dma_start(br_sb, br[:].rearrange("(c p) -> p c", p=128))
            bur_sb = consts.tile([128, MC], F32)
            nc.sync.dma_start(bur_sb, bur[:].rearrange("(c p) -> p c", p=128))

            hTf = consts.tile([128, MC, BL], F32)
            nc.vector.memset(hTf, 0.0)
            hTb = consts.tile([128, MC, BL], BF16)
            nc.vector.memset(hTb, 0.0)

            # ---------------- Phase 1: projections ----------------
            with (
                tc.tile_pool(name="proj_in", bufs=2) as pin,
                tc.tile_pool(name="proj_ps", bufs=4, space="PSUM") as pps,
                tc.tile_pool(name="proj_out", bufs=4) as pout,
            ):
                tblk = PCOLS // BL  # timesteps per column block
                assert tblk % S == 0 or S % tblk == 0
                cpb = max(1, tblk // S)  # swizzle chunks per column block
                for j in range(NBLK):
                    xT_st = pin.tile([128, KC, PCOLS], F32, tag="xT_st")
                    nc.sync.dma_start(
                        xT_st,
                        xT[:, j * PCOLS : (j + 1) * PCOLS].rearrange(
                            "(c p) n -> p c n", p=128
                        ),
                    )
                    xT_sb = pin.tile([128, KC, PCOLS], F32R, tag="xT_r")
                    nc.vector.tensor_copy(xT_sb, xT_st)
                    for W_sb, bias_sb, dst in (
                        (Wk_sb, br_sb, xzT_d),
                        (Wu_sb, bur_sb, xhT_d),
                    ):
                        for m in range(MC):
                            ps = pps.tile([128, PCOLS], F32)
                            for k in range(KC):
                                nc.tensor.matmul(
                                    ps,
                                    W_sb[:, k, m * 128 : (m + 1) * 128],
                                    xT_sb[:, k, :],
                                    start=(k == 0),
                                    stop=(k == KC - 1),
                                )
                            o = pout.tile([128, PCOLS], F32)
                            nc.scalar.activation(o, ps, ID, bias=bias_sb[:, m : m + 1])
                            if cpb >= 1 and tblk >= S:
                                nc.sync.dma_start(
                                    dst[j * cpb : (j + 1) * cpb, m, :, :].rearrange(
                                        "tc p n -> p tc n"
                                    ),
                                    o.rearrange("p (tc n) -> p tc n", tc=cpb),
                                )
                            else:  # S > tblk: one block fills part of a chunk
                                nc.sync.dma_start(
                                    dst[
                                        (j * tblk) // S,
                                        m,
                                        :,
                                        (j % (S // tblk)) * PCOLS : (j % (S // tblk))
                                        * PCOLS
                                        + PCOLS,
                                    ],
                                    o,
                                )

            # ---------------- Phase 2: recurrence ----------------
            dbg2 = os.environ.get("MGU_DEBUG2")
            if dbg2:
                f_dbg = nc.dram_tensor(
                    "f_dbg", [t_total, 128, MC, BL], F32, kind="ExternalOutput"
                )
                c_dbg = nc.dram_tensor(
                    "c_dbg", [t_total, 128, MC, BL], F32, kind="ExternalOutput"
                )
                h_dbg = nc.dram_tensor(
                    "h_dbg", [t_total, 128, MC, BL], F32, kind="ExternalOutput"
                )
            with (
                tc.tile_pool(name="rec_in", bufs=2) as rin,
                tc.tile_pool(name="rec_ps", bufs=2, space="PSUM") as rps,
                tc.tile_pool(name="rec_tmp", bufs=3) as rtmp,
            ):
                with tc.For_i(0, nch, 1, staggered_reset=True) as it:
                    xz_sb = rin.tile([128, 1, MC, NW], F32, tag="xz")
                    nc.sync.dma_start(
                        xz_sb,
                        xzT_d[bass.ds(it, 1), :, :, :].rearrange("o c p n -> p o c n"),
                    )
                    xh_sb = rin.tile([128, 1, MC, NW], F32, tag="xh")
                    nc.sync.dma_start(
                        xh_sb,
                        xhT_d[bass.ds(it, 1), :, :, :].rearrange("o c p n -> p o c n"),
                    )
                    for s in range(S):
                        bsl = slice(s * BL, (s + 1) * BL)
                        ps1 = rps.tile([128, MC, BL], F32, tag="ps1")
                        for m in range(MC):
                            for k in range(KC):
                                nc.tensor.matmul(
                                    ps1[:, m, :],
                                    Wr_sb[:, k, m * 128 : (m + 1) * 128],
                                    hTb[:, k, :],
                                    start=(k == 0),
                                    stop=(k == KC - 1),
                                )
                        fT = rtmp.tile([128, MC, BL], F32, tag="fT")
                        hfb = rtmp.tile([128, MC, BL], BF16, tag="hfb")
                        z1 = rtmp.tile([128, MC, BL], F32, tag="z1")
                        for m in range(MC):
                            nc.vector.tensor_add(
                                z1[:, m, :], ps1[:, m, :], xz_sb[:, 0, m, bsl]
                            )
                            nc.scalar.activation(fT[:, m, :], z1[:, m, :], SIG)
                            nc.vector.tensor_mul(
                                hfb[:, m, :], hTf[:, m, :], fT[:, m, :]
                            )
                        ps2 = rps.tile([128, MC, BL], F32, tag="ps2")
                        for m in range(MC):
                            for k in range(KC):
                                nc.tensor.matmul(
                                    ps2[:, m, :],
                                    Wur_sb[:, k, m * 128 : (m + 1) * 128],
                                    hfb[:, k, :],
                                    start=(k == 0),
                                    stop=(k == KC - 1),
                                )
                        cT = rtmp.tile([128, MC, BL], F32, tag="cT")
                        z2 = rtmp.tile([128, MC, BL], F32, tag="z2")
                        for m in range(MC):
                            nc.vector.tensor_add(
                                z2[:, m, :], ps2[:, m, :], xh_sb[:, 0, m, bsl]
                            )
                            nc.scalar.activation(cT[:, m, :], z2[:, m, :], TANH)
                        if dbg2:
                            nc.sync.dma_start(f_dbg[bass.ds(it * S + s, 1)], fT)
                            nc.sync.dma_start(c_dbg[bass.ds(it * S + s, 1)], cT)
                        nc.vector.tensor_sub(cT, cT, hTf)
                        nc.vector.tensor_mul(cT, cT, fT)
                        nc.vector.tensor_add(hTf, hTf, cT)
                        nc.scalar.copy(hTb, hTf)
                        if dbg2:
                            nc.sync.dma_start(h_dbg[bass.ds(it * S + s, 1)], hTf)

            nc.sync.dma_start(hT_out[:, :, :], hTf)

    nc.compile()
    return nc


_NC_CACHE = None


def kernel(x, W_k, W_r, b_r, W_u, W_ur, b_ur):
    global _NC_CACHE, LAST_EXEC_NS
    _install_trace_shim()
    if _NC_CACHE is None:
        _NC_CACHE = _build()
    nc = _NC_CACHE

    x = np.ascontiguousarray(np.asarray(x, dtype=np.float32))
    Wr_b = np.asarray(W_r, dtype=np.float32).astype(ml_dtypes.bfloat16)
    Wur_b = np.asarray(W_ur, dtype=np.float32).astype(ml_dtypes.bfloat16)
    Wk_f = np.ascontiguousarray(np.asarray(W_k, dtype=np.float32))
    Wu_f = np.ascontiguousarray(np.asarray(W_u, dtype=np.float32))
    br_f = np.ascontiguousarray(np.asarray(b_r, dtype=np.float32))
    bur_f = np.ascontiguousarray(np.asarray(b_ur, dtype=np.float32))

    in_maps = []
    for c in range(NCORES):
        xc = x[c * BL : (c + 1) * BL]  # [BL, T, D]
        xTc = np.ascontiguousarray(xc.transpose(2, 1, 0).reshape(D, T * BL))
        in_maps.append(
            {
                "xT": xTc,
                "Wk": Wk_f,
                "Wu": Wu_f,
                "Wr": Wr_b,
                "Wur": Wur_b,
                "br": br_f,
                "bur": bur_f,
            }
        )

    trace = bool(os.environ.get("BASS_TRACE"))
    res = run_bass_kernel_spmd(
        nc, in_maps, core_ids=list(range(NCORES)), trace=trace
    )
    LAST_EXEC_NS = res.exec_time_ns

    out = np.empty((B, U), dtype=np.float32)
    for c in range(NCORES):
        hT = res.results[c]["hT_out"]  # [128, MC, BL]
        out[c * BL : (c + 1) * BL] = hT.transpose(2, 1, 0).reshape(BL, U)
    return out


# revision 13
# speedup vs baseline: 1.8287x; 1.4339x over previous
"""Trainium2 Bass kernel for BasicMGU (nn_BasicMGU_53386443489965).

Math (per reference):
    xz = x @ W_k ; xh = x @ W_u
    f_t = sigmoid(xz_t + h @ W_r + b_r)
    c_t = tanh(xh_t + (h*f_t) @ W_ur + b_ur)
    h   = (1-f_t)*h + f_t*c_t        -> return final h  [B, U]

Sharding: data-parallel over batch across 8 cores (B=64 -> 8 per core),
weights replicated.

Per-core design:
  Phase 1 (projections): two GEMMs in fp32r (full PE rate at N=512),
  producing xzT/xhT in DRAM pre-swizzled into the exact per-chunk SBUF
  layout the recurrence consumes (contiguous 512B+ runs per DMA
  descriptor), biases folded in.
  Phase 2 (recurrence): state kept transposed hT [U(part), B(free)].
  Both per-step matmuls run weight-stationary (lhsT = 128x128 weight
  tile in bf16 -> fast weight load, rhs = state in bf16, N=B=8), so no
  per-step transposes are needed and PSUM outputs stay transposed.
  Accumulation groups are kept consecutive per PSUM slice (m-outer,
  k-inner) - interleaving groups gives wrong results on HW.
  Elementwise/activations run on [128, ...] tiles (128 partitions).
"""

import os
import sys
import types

sys.path.insert(0, "/opt/trn_rl_repo")

import numpy as np
import ml_dtypes

import concourse.bass as bass
import concourse.mybir as mybir
import concourse.tile as tile
from concourse import bacc
from concourse.bass_utils import run_bass_kernel_spmd

B, T, D, U = 64, 1024, 512, 512
NCORES = 8
BL = B // NCORES          # batch per core
S = int(os.environ.get("MGU_S", 32))  # recurrence steps per hw-loop iteration
KC = D // 128             # contraction chunks
MC = U // 128             # output-unit chunks
PCOLS = 512               # projection (t,b) columns per block
NBLK = T * BL // PCOLS
NW = S * BL               # free width of one swizzled chunk slab

F32 = mybir.dt.float32
F32R = mybir.dt.float32r
BF16 = mybir.dt.bfloat16

LAST_EXEC_NS = None


def _install_trace_shim():
    """Make `antenv.axon_hooks` importable so trace=True degrades gracefully
    (and, where the axon .so is present, actually captures NTFF profiles)."""
    if "antenv.axon_hooks" in sys.modules:
        return
    mod = types.ModuleType("antenv.axon_hooks")
    holder = [None]
    mod.set_axon_ntff_profile_hook = lambda h: holder.__setitem__(0, h)
    mod.get_axon_ntff_profile_hook = lambda: holder[0]
    sys.modules["antenv.axon_hooks"] = mod
    try:
        if "/root/.axon_site" not in sys.path:
            sys.path.append("/root/.axon_site")
        from trn_agent_boot.trn_boot import _ntff_profile_via_ctypes

        hook = _ntff_profile_via_ctypes("/opt/axon/libaxon_pjrt.so")
        if hook is not None:
            mod.set_axon_ntff_profile_hook(hook)
    except Exception:
        pass


def _build():
    nc = bacc.Bacc("TRN2")

    t_total = int(os.environ.get("MGU_TSTEPS", T))
    nch = t_total // S

    xT = nc.dram_tensor("xT", [D, T * BL], F32, kind="ExternalInput")
    Wk = nc.dram_tensor("Wk", [D, U], F32, kind="ExternalInput")
    Wu = nc.dram_tensor("Wu", [D, U], F32, kind="ExternalInput")
    Wr = nc.dram_tensor("Wr", [U, U], BF16, kind="ExternalInput")
    Wur = nc.dram_tensor("Wur", [U, U], BF16, kind="ExternalInput")
    br = nc.dram_tensor("br", [U], F32, kind="ExternalInput")
    bur = nc.dram_tensor("bur", [U], F32, kind="ExternalInput")
    hT_out = nc.dram_tensor("hT_out", [128, MC, BL], F32, kind="ExternalOutput")
    # Swizzled step-input slabs: [chunk, m, partition(u%128), (s b)]
    xzT_d = nc.dram_tensor("xzT_d", [T // S, MC, 128, NW], F32)
    xhT_d = nc.dram_tensor("xhT_d", [T // S, MC, 128, NW], F32)

    ID = mybir.ActivationFunctionType.Identity
    SIG = mybir.ActivationFunctionType.Sigmoid
    TANH = mybir.ActivationFunctionType.Tanh

    with tile.TileContext(nc) as tc:
        with tc.tile_pool(name="consts", bufs=1) as consts:
            # fp32r matmul inputs must be produced by a compute op (walrus
            # verifier rejects DMA-written fp32r operands), so stage via f32
            # tiles and round with a DVE copy.
            Wk_st = consts.tile([128, KC, U], F32)
            nc.sync.dma_start(Wk_st, Wk[:, :].rearrange("(c p) u -> p c u", p=128))
            Wk_sb = consts.tile([128, KC, U], F32R)
            nc.vector.tensor_copy(Wk_sb, Wk_st)
            Wu_st = consts.tile([128, KC, U], F32)
            nc.sync.dma_start(Wu_st, Wu[:, :].rearrange("(c p) u -> p c u", p=128))
            Wu_sb = consts.tile([128, KC, U], F32R)
            nc.vector.tensor_copy(Wu_sb, Wu_st)
            Wr_sb = consts.tile([128, MC, U], BF16)
            nc.sync.dma_start(Wr_sb, Wr[:, :].rearrange("(c p) u -> p c u", p=128))
            Wur_sb = consts.tile([128, MC, U], BF16)
            nc.sync.dma_start(Wur_sb, Wur[:, :].rearrange("(c p) u -> p c u", p=128))
            br_sb = consts.tile([128, MC], F32)
            nc.sync.dma_start(br_sb, br[:].rearrange("(c p) -> p c", p=128))
            bur_sb = consts.tile([128, MC], F32)
            nc.sync.dma_start(bur_sb, bur[:].rearrange("(c p) -> p c", p=128))

            hTf = consts.tile([128, MC, BL], F32)
            nc.vector.memset(hTf, 0.0)
            hTb = consts.tile([128, MC, BL], BF16)
            nc.vector.memset(hTb, 0.0)

            # ---------------- Phase 1: projections ----------------
            with (
                tc.tile_pool(name="proj_in", bufs=2) as pin,
                tc.tile_pool(name="proj_ps", bufs=4, space="PSUM") as pps,
                tc.tile_pool(name="proj_out", bufs=4) as pout,
            ):
                tblk = PCOLS // BL  # timesteps per column block
                assert tblk % S == 0 or S % tblk == 0
                cpb = max(1, tblk // S)  # swizzle chunks per column block
                for j in range(NBLK):
                    xT_st = pin.tile([128, KC, PCOLS], F32, tag="xT_st")
                    nc.sync.dma_start(
                        xT_st,
                        xT[:, j * PCOLS : (j + 1) * PCOLS].rearrange(
                            "(c p) n -> p c n", p=128
                        ),
                    )
                    xT_sb = pin.tile([128, KC, PCOLS], F32R, tag="xT_r")
                    nc.vector.tensor_copy(xT_sb, xT_st)
                    for W_sb, bias_sb, dst in (
                        (Wk_sb, br_sb, xzT_d),
                        (Wu_sb, bur_sb, xhT_d),
                    ):
                        for m in range(MC):
                            ps = pps.tile([128, PCOLS], F32)
                            for k in range(KC):
                                nc.tensor.matmul(
                                    ps,
                                    W_sb[:, k, m * 128 : (m + 1) * 128],
                                    xT_sb[:, k, :],
                                    start=(k == 0),
                                    stop=(k == KC - 1),
                                )
                            o = pout.tile([128, PCOLS], F32)
                            nc.scalar.activation(o, ps, ID, bias=bias_sb[:, m : m + 1])
                            if cpb >= 1 and tblk >= S:
                                nc.sync.dma_start(
                                    dst[j * cpb : (j + 1) * cpb, m, :, :].rearrange(
                                        "tc p n -> p tc n"
                                    ),
                                    o.rearrange("p (tc n) -> p tc n", tc=cpb),
                                )
                            else:  # S > tblk: one block fills part of a chunk
                                nc.sync.dma_start(
                                    dst[
                                        (j * tblk) // S,
                                        m,
                                        :,
                                        (j % (S // tblk)) * PCOLS : (j % (S // tblk))
                                        * PCOLS
                                        + PCOLS,
                                    ],
                                    o,
                                )

            # ---------------- Phase 2: recurrence ----------------
            dbg2 = os.environ.get("MGU_DEBUG2")
            if dbg2:
                f_dbg = nc.dram_tensor(
                    "f_dbg", [t_total, 128, MC, BL], F32, kind="ExternalOutput"
                )
                c_dbg = nc.dram_tensor(
                    "c_dbg", [t_total, 128, MC, BL], F32, kind="ExternalOutput"
                )
                h_dbg = nc.dram_tensor(
                    "h_dbg", [t_total, 128, MC, BL], F32, kind="ExternalOutput"
                )
            with (
                tc.tile_pool(name="rec_in", bufs=2) as rin,
                tc.tile_pool(name="rec_ps1", bufs=2, space="PSUM") as rps1,
                tc.tile_pool(name="rec_ps2", bufs=1, space="PSUM") as rps2,
                tc.tile_pool(name="rec_tmp", bufs=3) as rtmp,
            ):
                with tc.For_i(0, nch, 1, staggered_reset=True) as it:
                    xz_sb = rin.tile([128, 1, MC, NW], F32, tag="xz")
                    nc.sync.dma_start(
                        xz_sb,
                        xzT_d[bass.ds(it, 1), :, :, :].rearrange("o c p n -> p o c n"),
                    )
                    xh_sb = rin.tile([128, 1, MC, NW], F32, tag="xh")
                    nc.sync.dma_start(
                        xh_sb,
                        xhT_d[bass.ds(it, 1), :, :, :].rearrange("o c p n -> p o c n"),
                    )
                    for s in range(S):
                        bsl = slice(s * BL, (s + 1) * BL)
                        # mm1: psum pre-seeded with xz_t, accumulate h @ W_r
                        # on top (m-outer: consecutive accumulation groups
                        # per psum slice within one bank).
                        ps1 = rps1.tile([128, MC, BL], F32, tag="ps1")
                        nc.vector.tensor_copy(ps1, xz_sb[:, 0, :, bsl])
                        for m in range(MC):
                            for k in range(KC):
                                nc.tensor.matmul(
                                    ps1[:, m, :],
                                    Wr_sb[:, k, m * 128 : (m + 1) * 128],
                                    hTb[:, k, :],
                                    start=False,
                                    stop=(k == KC - 1),
                                )
                        fT = rtmp.tile([128, MC, BL], F32, tag="fT")
                        hfb = rtmp.tile([128, MC, BL], BF16, tag="hfb")
                        for m in range(MC):
                            nc.scalar.activation(fT[:, m, :], ps1[:, m, :], SIG)
                            nc.vector.tensor_mul(
                                hfb[:, m, :], hTf[:, m, :], fT[:, m, :]
                            )
                        # off critical path: A = h - h*f (exact, fp32)
                        hf32 = rtmp.tile([128, MC, BL], F32, tag="hf32")
                        nc.vector.tensor_mul(hf32, hTf, fT)
                        A = rtmp.tile([128, MC, BL], F32, tag="A")
                        nc.vector.tensor_sub(A, hTf, hf32)
                        # mm2: 4 separate PSUM banks (one per m) so k-outer
                        # issue order (follows hfb chunk availability) keeps
                        # accumulation groups per-bank.
                        ps2 = rps2.tile([128, MC, 512], F32, tag="ps2")
                        nc.vector.tensor_copy(ps2[:, :, 0:BL], xh_sb[:, 0, :, bsl])
                        for k in range(KC):
                            for m in range(MC):
                                nc.tensor.matmul(
                                    ps2[:, m, 0:BL],
                                    Wur_sb[:, k, m * 128 : (m + 1) * 128],
                                    hfb[:, k, :],
                                    start=False,
                                    stop=(k == KC - 1),
                                )
                        cT = rtmp.tile([128, MC, BL], F32, tag="cT")
                        nc.scalar.activation(cT, ps2[:, :, 0:BL], TANH)
                        if dbg2:
                            nc.sync.dma_start(f_dbg[bass.ds(it * S + s, 1)], fT)
                            nc.sync.dma_start(c_dbg[bass.ds(it * S + s, 1)], cT)
                        # chain: e = f*c ; h_bf16 = A + e first (unblocks next
                        # step's mm1), fp32 master update shadows it.
                        nc.vector.tensor_mul(cT, cT, fT)
                        nc.vector.tensor_add(hTb, A, cT)
                        nc.vector.tensor_add(hTf, A, cT)
                        if dbg2:
                            nc.sync.dma_start(h_dbg[bass.ds(it * S + s, 1)], hTf)

            nc.sync.dma_start(hT_out[:, :, :], hTf)

    nc.compile()
    return nc


_NC_CACHE = None


def kernel(x, W_k, W_r, b_r, W_u, W_ur, b_ur):
    global _NC_CACHE, LAST_EXEC_NS
    _install_trace_shim()
    if _NC_CACHE is None:
        _NC_CACHE = _build()
    nc = _NC_CACHE

    x = np.ascontiguousarray(np.asarray(x, dtype=np.float32))
    Wr_b = np.asarray(W_r, dtype=np.float32).astype(ml_dtypes.bfloat16)
    Wur_b = np.asarray(W_ur, dtype=np.float32).astype(ml_dtypes.bfloat16)
    Wk_f = np.ascontiguousarray(np.asarray(W_k, dtype=np.float32))
    Wu_f = np.ascontiguousarray(np.asarray(W_u, dtype=np.float32))
    br_f = np.ascontiguousarray(np.asarray(b_r, dtype=np.float32))
    bur_f = np.ascontiguousarray(np.asarray(b_ur, dtype=np.float32))

    in_maps = []
    for c in range(NCORES):
        xc = x[c * BL : (c + 1) * BL]  # [BL, T, D]
        xTc = np.ascontiguousarray(xc.transpose(2, 1, 0).reshape(D, T * BL))
        in_maps.append(
            {
                "xT": xTc,
                "Wk": Wk_f,
                "Wu": Wu_f,
                "Wr": Wr_b,
                "Wur": Wur_b,
                "br": br_f,
                "bur": bur_f,
            }
        )

    trace = bool(os.environ.get("BASS_TRACE"))
    res = run_bass_kernel_spmd(
        nc, in_maps, core_ids=list(range(NCORES)), trace=trace
    )
    LAST_EXEC_NS = res.exec_time_ns

    out = np.empty((B, U), dtype=np.float32)
    for c in range(NCORES):
        hT = res.results[c]["hT_out"]  # [128, MC, BL]
        out[c * BL : (c + 1) * BL] = hT.transpose(2, 1, 0).reshape(BL, U)
    return out


# revision 16
# speedup vs baseline: 1.9886x; 1.0874x over previous
"""Trainium2 Bass kernel for BasicMGU (nn_BasicMGU_53386443489965).

Math (per reference):
    xz = x @ W_k ; xh = x @ W_u
    f_t = sigmoid(xz_t + h @ W_r + b_r)
    c_t = tanh(xh_t + (h*f_t) @ W_ur + b_ur)
    h   = (1-f_t)*h + f_t*c_t        -> return final h  [B, U]

Sharding: data-parallel over batch across 8 cores (B=64 -> 8 per core),
weights replicated.

Per-core design:
  Phase 1 (projections): two GEMMs in fp32r (full PE rate at N=512),
  producing xzT/xhT in DRAM pre-swizzled into the exact per-chunk SBUF
  layout the recurrence consumes (contiguous 512B+ runs per DMA
  descriptor), biases folded in.
  Phase 2 (recurrence): state kept transposed hT [U(part), B(free)].
  Both per-step matmuls run weight-stationary (lhsT = 128x128 weight
  tile in bf16 -> fast weight load, rhs = state in bf16, N=B=8), so no
  per-step transposes are needed and PSUM outputs stay transposed.
  Accumulation groups are kept consecutive per PSUM slice (m-outer,
  k-inner) - interleaving groups gives wrong results on HW.
  Elementwise/activations run on [128, ...] tiles (128 partitions).
"""

import os
import sys
import types

sys.path.insert(0, "/opt/trn_rl_repo")

import numpy as np
import ml_dtypes

import concourse.bass as bass
import concourse.mybir as mybir
import concourse.tile as tile
from concourse import bacc
from concourse.bass_utils import run_bass_kernel_spmd

B, T, D, U = 64, 1024, 512, 512
NCORES = 8
BL = B // NCORES          # batch per core
S = int(os.environ.get("MGU_S", 32))  # recurrence steps per hw-loop iteration
KC = D // 128             # contraction chunks
MC = U // 128             # output-unit chunks
PCOLS = 512               # projection (t,b) columns per block
NBLK = T * BL // PCOLS
NW = S * BL               # free width of one swizzled chunk slab

F32 = mybir.dt.float32
F32R = mybir.dt.float32r
BF16 = mybir.dt.bfloat16

LAST_EXEC_NS = None


def _install_trace_shim():
    """Make `antenv.axon_hooks` importable so trace=True degrades gracefully
    (and, where the axon .so is present, actually captures NTFF profiles)."""
    if "antenv.axon_hooks" in sys.modules:
        return
    mod = types.ModuleType("antenv.axon_hooks")
    holder = [None]
    mod.set_axon_ntff_profile_hook = lambda h: holder.__setitem__(0, h)
    mod.get_axon_ntff_profile_hook = lambda: holder[0]
    sys.modules["antenv.axon_hooks"] = mod
    try:
        if "/root/.axon_site" not in sys.path:
            sys.path.append("/root/.axon_site")
        from trn_agent_boot.trn_boot import _ntff_profile_via_ctypes

        hook = _ntff_profile_via_ctypes("/opt/axon/libaxon_pjrt.so")
        if hook is not None:
            mod.set_axon_ntff_profile_hook(hook)
    except Exception:
        pass


if os.environ.get("MGU_LDWOPT"):
    import concourse.bass_utils as _bu

    _orig_run_command = _bu.run_command

    def _run_command_ldwopt(argv, **kw):
        argv = [
            a.replace("--enable-ldw-opt=false", "--enable-ldw-opt=true")
            for a in argv
        ]
        return _orig_run_command(argv, **kw)

    _bu.run_command = _run_command_ldwopt


def _build():
    nc = bacc.Bacc("TRN2")

    t_total = int(os.environ.get("MGU_TSTEPS", T))
    nch = t_total // S

    xT = nc.dram_tensor("xT", [D, T * BL], F32, kind="ExternalInput")
    Wk = nc.dram_tensor("Wk", [D, U], F32, kind="ExternalInput")
    Wu = nc.dram_tensor("Wu", [D, U], F32, kind="ExternalInput")
    Wr = nc.dram_tensor("Wr", [U, U], BF16, kind="ExternalInput")
    Wur = nc.dram_tensor("Wur", [U, U], BF16, kind="ExternalInput")
    br = nc.dram_tensor("br", [U], F32, kind="ExternalInput")
    bur = nc.dram_tensor("bur", [U], F32, kind="ExternalInput")
    hT_out = nc.dram_tensor("hT_out", [128, MC, BL], F32, kind="ExternalOutput")
    # Swizzled step-input slabs: [chunk, m, partition(u%128), (s b)]
    xzT_d = nc.dram_tensor("xzT_d", [T // S, MC, 128, NW], F32)
    xhT_d = nc.dram_tensor("xhT_d", [T // S, MC, 128, NW], F32)

    ID = mybir.ActivationFunctionType.Identity
    SIG = mybir.ActivationFunctionType.Sigmoid
    TANH = mybir.ActivationFunctionType.Tanh

    with tile.TileContext(nc) as tc:
        with tc.tile_pool(name="consts", bufs=1) as consts:
            # fp32r matmul inputs must be produced by a compute op (walrus
            # verifier rejects DMA-written fp32r operands), so stage via f32
            # tiles and round with a DVE copy.
            Wk_st = consts.tile([128, KC, U], F32)
            nc.sync.dma_start(Wk_st, Wk[:, :].rearrange("(c p) u -> p c u", p=128))
            Wk_sb = consts.tile([128, KC, U], F32R)
            nc.vector.tensor_copy(Wk_sb, Wk_st)
            Wu_st = consts.tile([128, KC, U], F32)
            nc.sync.dma_start(Wu_st, Wu[:, :].rearrange("(c p) u -> p c u", p=128))
            Wu_sb = consts.tile([128, KC, U], F32R)
            nc.vector.tensor_copy(Wu_sb, Wu_st)
            Wr_sb = consts.tile([128, MC, U], BF16)
            nc.sync.dma_start(Wr_sb, Wr[:, :].rearrange("(c p) u -> p c u", p=128))
            Wur_sb = consts.tile([128, MC, U], BF16)
            nc.sync.dma_start(Wur_sb, Wur[:, :].rearrange("(c p) u -> p c u", p=128))
            br_sb = consts.tile([128, MC], F32)
            nc.sync.dma_start(br_sb, br[:].rearrange("(c p) -> p c", p=128))
            bur_sb = consts.tile([128, MC], F32)
            nc.sync.dma_start(bur_sb, bur[:].rearrange("(c p) -> p c", p=128))

            hTf = consts.tile([128, MC, BL], F32)
            nc.vector.memset(hTf, 0.0)
            hTb = consts.tile([128, MC, BL], BF16)
            nc.vector.memset(hTb, 0.0)

            # ---------------- Phase 1: projections ----------------
            with (
                tc.tile_pool(name="proj_in", bufs=2) as pin,
                tc.tile_pool(name="proj_ps", bufs=4, space="PSUM") as pps,
                tc.tile_pool(name="proj_out", bufs=4) as pout,
            ):
                tblk = PCOLS // BL  # timesteps per column block
                assert tblk % S == 0 or S % tblk == 0
                cpb = max(1, tblk // S)  # swizzle chunks per column block
                for j in range(NBLK):
                    xT_st = pin.tile([128, KC, PCOLS], F32, tag="xT_st")
                    nc.sync.dma_start(
                        xT_st,
                        xT[:, j * PCOLS : (j + 1) * PCOLS].rearrange(
                            "(c p) n -> p c n", p=128
                        ),
                    )
                    xT_sb = pin.tile([128, KC, PCOLS], F32R, tag="xT_r")
                    nc.vector.tensor_copy(xT_sb, xT_st)
                    for W_sb, bias_sb, dst in (
                        (Wk_sb, br_sb, xzT_d),
                        (Wu_sb, bur_sb, xhT_d),
                    ):
                        for m in range(MC):
                            ps = pps.tile([128, PCOLS], F32)
                            for k in range(KC):
                                nc.tensor.matmul(
                                    ps,
                                    W_sb[:, k, m * 128 : (m + 1) * 128],
                                    xT_sb[:, k, :],
                                    start=(k == 0),
                                    stop=(k == KC - 1),
                                )
                            o = pout.tile([128, PCOLS], F32)
                            nc.scalar.activation(o, ps, ID, bias=bias_sb[:, m : m + 1])
                            if cpb >= 1 and tblk >= S:
                                nc.sync.dma_start(
                                    dst[j * cpb : (j + 1) * cpb, m, :, :].rearrange(
                                        "tc p n -> p tc n"
                                    ),
                                    o.rearrange("p (tc n) -> p tc n", tc=cpb),
                                )
                            else:  # S > tblk: one block fills part of a chunk
                                nc.sync.dma_start(
                                    dst[
                                        (j * tblk) // S,
                                        m,
                                        :,
                                        (j % (S // tblk)) * PCOLS : (j % (S // tblk))
                                        * PCOLS
                                        + PCOLS,
                                    ],
                                    o,
                                )

            # ---------------- Phase 2: recurrence ----------------
            dbg2 = os.environ.get("MGU_DEBUG2")
            if dbg2:
                f_dbg = nc.dram_tensor(
                    "f_dbg", [t_total, 128, MC, BL], F32, kind="ExternalOutput"
                )
                c_dbg = nc.dram_tensor(
                    "c_dbg", [t_total, 128, MC, BL], F32, kind="ExternalOutput"
                )
                h_dbg = nc.dram_tensor(
                    "h_dbg", [t_total, 128, MC, BL], F32, kind="ExternalOutput"
                )
            with (
                tc.tile_pool(name="rec_in", bufs=2) as rin,
                tc.tile_pool(name="rec_ps1", bufs=2, space="PSUM") as rps1,
                tc.tile_pool(name="rec_ps2", bufs=2, space="PSUM") as rps2,
                tc.tile_pool(name="rec_tmp", bufs=3) as rtmp,
            ):
                with tc.For_i(0, nch, 1, staggered_reset=True) as it:
                    xz_sb = rin.tile([128, 1, MC, NW], F32, tag="xz")
                    nc.sync.dma_start(
                        xz_sb,
                        xzT_d[bass.ds(it, 1), :, :, :].rearrange("o c p n -> p o c n"),
                    )
                    xh_sb = rin.tile([128, 1, MC, NW], F32, tag="xh")
                    nc.sync.dma_start(
                        xh_sb,
                        xhT_d[bass.ds(it, 1), :, :, :].rearrange("o c p n -> p o c n"),
                    )
                    for s in range(S):
                        bsl = slice(s * BL, (s + 1) * BL)
                        # mm1: psum pre-seeded with xz_t, accumulate h @ W_r
                        # on top (m-outer: consecutive accumulation groups
                        # per psum slice within one bank).
                        ps1 = rps1.tile([128, MC, BL], F32, tag="ps1")
                        nc.vector.tensor_copy(ps1, xz_sb[:, 0, :, bsl])
                        for m in range(MC):
                            for k in range(KC):
                                nc.tensor.matmul(
                                    ps1[:, m, :],
                                    Wr_sb[:, k, m * 128 : (m + 1) * 128],
                                    hTb[:, k, :],
                                    start=False,
                                    stop=(k == KC - 1),
                                )
                        fT = rtmp.tile([128, MC, BL], F32, tag="fT")
                        nc.scalar.activation(fT, ps1, SIG)
                        hfb = rtmp.tile([128, MC, BL], BF16, tag="hfb")
                        nc.vector.tensor_mul(hfb, hTf, fT)
                        # off critical path: A = h - h*f (exact, fp32)
                        hf32 = rtmp.tile([128, MC, BL], F32, tag="hf32")
                        nc.vector.tensor_mul(hf32, hTf, fT)
                        A = rtmp.tile([128, MC, BL], F32, tag="A")
                        nc.vector.tensor_sub(A, hTf, hf32)
                        ps2 = rps2.tile([128, MC, BL], F32, tag="ps2")
                        nc.vector.tensor_copy(ps2, xh_sb[:, 0, :, bsl])
                        for m in range(MC):
                            for k in range(KC):
                                nc.tensor.matmul(
                                    ps2[:, m, :],
                                    Wur_sb[:, k, m * 128 : (m + 1) * 128],
                                    hfb[:, k, :],
                                    start=False,
                                    stop=(k == KC - 1),
                                )
                        cT = rtmp.tile([128, MC, BL], F32, tag="cT")
                        nc.scalar.activation(cT, ps2, TANH)
                        if dbg2:
                            nc.sync.dma_start(f_dbg[bass.ds(it * S + s, 1)], fT)
                            nc.sync.dma_start(c_dbg[bass.ds(it * S + s, 1)], cT)
                        # chain: e = f*c ; h_bf16 = A + e first (unblocks next
                        # step's mm1), fp32 master update shadows it.
                        nc.vector.tensor_mul(cT, cT, fT)
                        nc.vector.tensor_add(hTb, A, cT)
                        nc.vector.tensor_add(hTf, A, cT)
                        if dbg2:
                            nc.sync.dma_start(h_dbg[bass.ds(it * S + s, 1)], hTf)

            nc.sync.dma_start(hT_out[:, :, :], hTf)

    nc.compile()
    return nc


_NC_CACHE = None


def kernel(x, W_k, W_r, b_r, W_u, W_ur, b_ur):
    global _NC_CACHE, LAST_EXEC_NS
    _install_trace_shim()
    if _NC_CACHE is None:
        _NC_CACHE = _build()
    nc = _NC_CACHE

    x = np.ascontiguousarray(np.asarray(x, dtype=np.float32))
    Wr_b = np.asarray(W_r, dtype=np.float32).astype(ml_dtypes.bfloat16)
    Wur_b = np.asarray(W_ur, dtype=np.float32).astype(ml_dtypes.bfloat16)
    Wk_f = np.ascontiguousarray(np.asarray(W_k, dtype=np.float32))
    Wu_f = np.ascontiguousarray(np.asarray(W_u, dtype=np.float32))
    br_f = np.ascontiguousarray(np.asarray(b_r, dtype=np.float32))
    bur_f = np.ascontiguousarray(np.asarray(b_ur, dtype=np.float32))

    in_maps = []
    for c in range(NCORES):
        xc = x[c * BL : (c + 1) * BL]  # [BL, T, D]
        xTc = np.ascontiguousarray(xc.transpose(2, 1, 0).reshape(D, T * BL))
        in_maps.append(
            {
                "xT": xTc,
                "Wk": Wk_f,
                "Wu": Wu_f,
                "Wr": Wr_b,
                "Wur": Wur_b,
                "br": br_f,
                "bur": bur_f,
            }
        )

    trace = bool(os.environ.get("BASS_TRACE"))
    res = run_bass_kernel_spmd(
        nc, in_maps, core_ids=list(range(NCORES)), trace=trace
    )
    LAST_EXEC_NS = res.exec_time_ns

    out = np.empty((B, U), dtype=np.float32)
    for c in range(NCORES):
        hT = res.results[c]["hT_out"]  # [128, MC, BL]
        out[c * BL : (c + 1) * BL] = hT.transpose(2, 1, 0).reshape(BL, U)
    return out
